# revision 57
# baseline (speedup 1.0000x reference)
"""Multi-head self-attention (B=4, S=2048, D=1024, H=16) on 8 TRN2 NeuronCores.

Sharding: data-parallel over batch x tensor-parallel over heads (Megatron
column-split of w_qkv, row-split of w_out). Core c computes batch c//2 with
heads (c%2)*8..(c%2)*8+8 and produces a partial [S, D] output; the host sums
the two partials per batch and adds the bias.

Per-core kernel (all-bf16 matmuls, fp32 PSUM accumulate, ~5e-3 rel err):
  - x rows are DMA'd in, cast to bf16 on DVE, PE-transposed (bf16, 1 cyc/row,
    back-to-back transposes pipeline at ~56 ns) into a resident d-major xT
    SBUF tile [128, 8*2048] — no DRAM staging round-trip; the (0,0) attention
    pass starts incrementally as each 512-key chunk lands.
  - v = x @ wv kept seq-major in SBUF with a ones column per head (softmax
    denominators); qT/kT computed feat-major per head-pair [128, S] just in
    time, overlapped with the previous pair's attention.
  - merged-parity passes (ft, qh of 512 queries): per k-tile the two heads'
    QK matmuls (K=64 each) go to disjoint PE row-groups (rows 0-63 / 64-127,
    dstart ~4 ns — concurrent on the 16x32x32 sub-array grid) and QKs of two
    consecutive k-tiles are emitted adjacently so same-row-group MMs pipeline
    past the pair's drain; both halves land in one [128, 1024] fp32 PSUM tile
    read by a SINGLE exp ACTIVATE (1113 ns, the per-k-tile pace setter).
  - PV per parity accumulates [65, 512] fp32 PSUM one k-tile behind QK.
    PSUM budget is exactly 8 banks: sc 2x2 + pv 2x1 + proj/v/y ring 2x1.
  - normalize: per-pass denominators are bounced through DRAM [1,512]x2 ->
    [8,128] so the slow DVE reciprocal (7.8 ns/elem/lane) runs on 8 lanes,
    then DMA'd back, partition-broadcast (GPSIMD) and multiplied into
    feat-major outP tiles — all deferred into the next pass.
  - y = sum_ft outP^T @ wout at K=128, interleaved with the last pair; the
    final pass normalizes and emits y per 128-query tile so the tail
    pipelines across DVE/GPSIMD/PE instead of one serial chain.
"""

import numpy as np

from concourse import bass_utils

from contextlib import ExitStack

import concourse.bacc as bacc
import concourse.bass as bass
import concourse.mybir as mybir
import concourse.tile as tile
from concourse import masks

P = 128
HD = 64
HV = HD + 1
QCH = 512
F32 = mybir.dt.float32
BF16 = mybir.dt.bfloat16
EXP = mybir.ActivationFunctionType.Exp


def build_attention_v2(
    S: int, D: int, HN: int, DO: int, scale: float
) -> bacc.Bacc:
    F = HN * HD
    n_st = S // P
    n_dt = D // P
    n_ft = F // P
    n_ch = S // QCH
    n_kt = S // P
    n_qh = S // QCH
    n_no = DO // QCH
    n_sti = QCH // P
    assert S % QCH == 0 and D % P == 0 and F % P == 0 and DO % QCH == 0

    nc = bacc.Bacc("TRN2", target_bir_lowering=False, debug=False)

    x = nc.dram_tensor("x", [S, D], F32, kind="ExternalInput")
    wq = nc.dram_tensor("wq", [D, F], BF16, kind="ExternalInput")
    wk = nc.dram_tensor("wk", [D, F], BF16, kind="ExternalInput")
    wv = nc.dram_tensor("wv", [D, F], BF16, kind="ExternalInput")
    wout = nc.dram_tensor("wout", [F, DO], BF16, kind="ExternalInput")
    y = nc.dram_tensor("y", [S, DO], F32, kind="ExternalOutput")
    # scratch for the batched-reciprocal partition reshape (DRAM bounce:
    # SBUF APs cannot split one partition's row across partitions)
    den_scr = nc.dram_tensor("den_scr", [n_ft * (S // QCH), 2, QCH], F32)
    rec_scr = nc.dram_tensor("rec_scr", [n_ft * (S // QCH), 2, QCH], F32)

    with tile.TileContext(nc) as tc, ExitStack() as top:  # noqa: PLR1702
        const_pool = top.enter_context(tc.tile_pool(name="const", bufs=1))
        ones_f32 = const_pool.tile([P, HD], F32, tag="ones_f32")
        nc.gpsimd.memset(ones_f32[:], 1.0)
        ident = const_pool.tile([P, P], F32, tag="ident")
        masks.make_identity(nc, ident[:])
        ident_b = const_pool.tile([P, P], BF16, tag="identb")
        nc.vector.tensor_copy(ident_b[:], ident[:])

        v_pool = top.enter_context(tc.tile_pool(name="vsb", bufs=1))
        v_sb = [
            v_pool.tile([P, HN * HV], BF16, tag=f"v{st}", name=f"v_sb{st}")
            for st in range(n_st)
        ]
        for st in range(n_st):
            nc.vector.tensor_copy(
                v_sb[st][:].rearrange("p (h v) -> p h v", v=HV)[:, :, HD:].rearrange(
                    "p h one -> p (h one)"
                ),
                ones_f32[:, :HN],
            )

        outT_pool = top.enter_context(tc.tile_pool(name="outT", bufs=1))
        outP = [
            outT_pool.tile([P, S], BF16, tag=f"o{ft}", name=f"outP{ft}")
            for ft in range(n_ft)
        ]

        # resident d-major xT: [128, db*S + s] bf16 (32 KB/partition)
        xT_pool = top.enter_context(tc.tile_pool(name="xT", bufs=1))
        xT_all = xT_pool.tile([P, n_dt * S], BF16, tag="xT", name="xT_all")

        def xT_sl(db, c0, c1):
            return xT_all[:, db * S + c0 : db * S + c1]

        wqk_pool = top.enter_context(tc.tile_pool(name="wqk", bufs=1))
        wq_s = [
            wqk_pool.tile([P, n_dt * P], BF16, tag=f"wqs{ft}", name=f"wqs{ft}")
            for ft in range(n_ft)
        ]
        wk_s = [
            wqk_pool.tile([P, n_dt * P], BF16, tag=f"wks{ft}", name=f"wks{ft}")
            for ft in range(n_ft)
        ]
        wq_t = [[wq_s[ft][:, db * P : (db + 1) * P] for ft in range(n_ft)] for db in range(n_dt)]
        wk_t = [[wk_s[ft][:, db * P : (db + 1) * P] for ft in range(n_ft)] for db in range(n_dt)]

        def load_w_pair(ft):
            nc.gpsimd.dma_start(
                wq_s[ft][:].rearrange("p (db c) -> p db c", c=P),
                wq[:, ft * P : (ft + 1) * P].rearrange("(db p) c -> p db c", p=P),
            )
            nc.gpsimd.dma_start(
                wk_s[ft][:].rearrange("p (db c) -> p db c", c=P),
                wk[:, ft * P : (ft + 1) * P].rearrange("(db p) c -> p db c", p=P),
            )

        wv_pool = top.enter_context(tc.tile_pool(name="wvp", bufs=1))
        wv_t = [
            wv_pool.tile([P, F], BF16, tag=f"wv{db}", name=f"wv{db}")
            for db in range(n_dt)
        ]

        pair_pool = top.enter_context(tc.tile_pool(name="pair", bufs=2))
        pair_tiles = {}

        def get_pair(ft):
            if ft not in pair_tiles:
                pair_tiles[ft] = (
                    pair_pool.tile([P, S], BF16, tag="qp", name=f"qTp{ft}"),
                    pair_pool.tile([P, S], BF16, tag="kp", name=f"kTp{ft}"),
                )
            return pair_tiles[ft]

        # sbuf working pools (xst stays open — pool release is LIFO-only)
        xst_pool = top.enter_context(tc.tile_pool(name="xst", bufs=2 * n_sti))

        # chunk-0 x rows issue FIRST (alternating queues) so the attention
        # ramp is not queued behind ~1.3 MB of weight DMA traffic
        preloaded = {}
        for sti in range(n_sti):
            xrow = xst_pool.tile([P, D], F32, tag="xrow", name=f"xrow{sti}")
            eng = nc.sync if sti % 2 == 0 else nc.gpsimd
            eng.dma_start(xrow[:], x[sti * P : (sti + 1) * P, :])
            preloaded[sti] = xrow
        load_w_pair(0)
        for db in range(n_dt):
            nc.gpsimd.dma_start(wv_t[db][:], wv[db * P : (db + 1) * P, :])
        e_pool = top.enter_context(tc.tile_pool(name="epool", bufs=6))
        stg_pool = top.enter_context(tc.tile_pool(name="stgpool", bufs=4))
        rc_pool = top.enter_context(tc.tile_pool(name="rcpool", bufs=2))
        bcs_pool = top.enter_context(tc.tile_pool(name="bcspool", bufs=2))

        # PSUM: sc 2x2 banks + pv 2x1 + misc 2x1 = 8 banks exactly
        ps_sc = top.enter_context(
            tc.tile_pool(name="ps_sc", bufs=2, space=bass.MemorySpace.PSUM)
        )
        ps_pv = top.enter_context(
            tc.tile_pool(name="ps_pv", bufs=2, space=bass.MemorySpace.PSUM)
        )
        ps_mp = top.enter_context(
            tc.tile_pool(name="ps_mp", bufs=2, space=bass.MemorySpace.PSUM)
        )

        # ---------------- building blocks ----------------
        def upfront_loads(ch):
            """DMA x rows (alternating queues so transfers overlap), cast to
            bf16, PE-transpose into resident xT."""
            xbs = []
            for sti in range(n_sti):
                st = ch * n_sti + sti
                if st in preloaded:
                    xrow = preloaded[st]
                else:
                    xrow = xst_pool.tile([P, D], F32, tag="xrow", name=f"xrow{st}")
                    eng = nc.sync if sti % 2 == 0 else nc.gpsimd
                    eng.dma_start(xrow[:], x[st * P : (st + 1) * P, :])
                xb = xst_pool.tile([P, D], BF16, tag="xbf", name=f"xb{st}")
                nc.vector.tensor_copy(xb[:], xrow[:])
                xbs.append(xb)
            for db in range(n_dt):
                tp = ps_mp.tile([P, QCH], BF16, tag="mp", name=f"tr{ch}_{db}")
                for sti in range(n_sti):
                    nc.tensor.transpose(
                        tp[:, sti * P : (sti + 1) * P],
                        xbs[sti][:, db * P : (db + 1) * P],
                        ident_b[:],
                    )
                nc.vector.tensor_copy(
                    xT_sl(db, ch * QCH, (ch + 1) * QCH), tp[:]
                )

        def proj_item(ftn, ch, w_t, dstp, which):
            def run():
                pp = ps_mp.tile([P, QCH], F32, tag="mp", name=f"pj{which}{ftn}_{ch}")
                for db in range(n_dt):
                    nc.tensor.matmul(
                        pp[:],
                        w_t[db][ftn],
                        xT_sl(db, ch * QCH, (ch + 1) * QCH),
                        start=(db == 0),
                        stop=(db == n_dt - 1),
                    )
                nc.vector.tensor_copy(dstp[:, ch * QCH : (ch + 1) * QCH], pp[:])

            return run

        def v_item(st):
            def run():
                pv_ps = ps_mp.tile([P, F], F32, tag="mp", name=f"pvp{st}")
                for db in range(n_dt):
                    nc.tensor.matmul(
                        pv_ps[:],
                        xT_sl(db, st * P, (st + 1) * P),
                        wv_t[db][:],
                        start=(db == 0),
                        stop=(db == n_dt - 1),
                    )
                nc.vector.tensor_copy(
                    v_sb[st][:].rearrange("p (h v) -> p h v", v=HV)[:, :, :HD],
                    pv_ps[:].rearrange("p (h d) -> p h d", d=HD),
                )

            return run

        class MergedPass:
            """One (ft, qh) pass: both head-parities per k-tile. QK_A/QK_B go
            to disjoint PE row-groups (concurrent); one exp ACTIVATE covers
            both; PV per parity runs one k-tile behind."""

            def __init__(self, ft, qh):
                self.ft, self.qh = ft, qh
                self.qTp, self.kTp = get_pair(ft)
                self.q0 = qh * QCH
                self.pv = [
                    ps_pv.tile([HV, QCH], F32, tag="pv", name=f"pv{ft}_{qh}_{p}")
                    for p in (0, 1)
                ]
                self.prev = None

            def _qk(self, kt):
                """QK for both parities: disjoint row-groups, concurrent."""
                ft, qh, q0 = self.ft, self.qh, self.q0
                sc = ps_sc.tile([P, 2 * QCH], F32, tag="sc", name=f"sc{ft}{qh}{kt}")
                for par in (0, 1):
                    sub = par * HD
                    nc.tensor.matmul(
                        sc[:, par * QCH : (par + 1) * QCH],
                        self.kTp[sub : sub + HD, kt * P : (kt + 1) * P],
                        self.qTp[sub : sub + HD, q0 : q0 + QCH],
                        start=True,
                        stop=True,
                    )
                et = e_pool.tile([P, 2 * QCH], BF16, tag="et", name=f"e{ft}{qh}{kt}")
                nc.scalar.activation(et[:], sc[:], EXP, scale=scale)
                return kt, et

            def emit_kts(self, kts, sprinkle=None, stride=1):
                # QKs for two k-tiles issue back-to-back: a full-array MM after
                # a row-paired QK waits for the pair's drain, but same-row-group
                # QKs pipeline, so batching 2 kts amortizes the drain stall
                kts = list(kts)
                idx = 0
                for g in range(0, len(kts), 2):
                    pair = kts[g : g + 2]
                    # sprinkle only at group boundaries so no full-array MM
                    # lands between the pipelined QK pair
                    for _ in pair:
                        if sprinkle and idx % stride == 0:
                            sprinkle.pop(0)()
                        idx += 1
                    new = [self._qk(kt) for kt in pair]
                    for item in new:
                        if self.prev is not None:
                            self._pv(self.prev)
                        self.prev = item

            def _pv(self, prev):
                kt, et = prev
                for par in (0, 1):
                    h = 2 * self.ft + par
                    vt = v_sb[kt][:].rearrange("p (hh v) -> p hh v", v=HV)[:, h, :]
                    nc.tensor.matmul(
                        self.pv[par][:, :],
                        vt,
                        et[:, par * QCH : (par + 1) * QCH],
                        start=(kt == 0),
                        stop=(kt == n_kt - 1),
                    )

            def finish_stage1(self):
                self._pv(self.prev)
                ft, qh = self.ft, self.qh
                self.stg = [
                    stg_pool.tile([HV, QCH], F32, tag="stg", name=f"st{ft}{qh}{p}")
                    for p in (0, 1)
                ]
                for par in (0, 1):
                    nc.vector.tensor_copy(self.stg[par][:], self.pv[par][:])

            def normalize_items(self):
                """Batched reciprocal: both parities' [1,512] denominator rows
                DMA-reshaped into one [8,128] tile so the slow DVE reciprocal
                uses 8 lanes, then DMA'd back and broadcast+multiplied."""
                ft, qh, q0 = self.ft, self.qh, self.q0
                self.rc = [None, None]

                pi = ft * n_qh + qh

                def recip_item():
                    # bounce both parities' [1,512] denominator rows through
                    # DRAM into [8,128] so the slow DVE reciprocal runs on 8
                    # lanes (~1us instead of 2x4us)
                    for par in (0, 1):
                        nc.sync.dma_start(
                            den_scr[pi, par : par + 1, :],
                            self.stg[par][HD : HD + 1, :],
                        )
                    den8 = rc_pool.tile([8, P], F32, tag="d8", name=f"d8{ft}{qh}")
                    nc.sync.dma_start(
                        den8[:],
                        den_scr[pi].rearrange("two (a c) -> (two a) c", c=P),
                    )
                    rec8 = rc_pool.tile([8, P], F32, tag="r8", name=f"r8{ft}{qh}")
                    nc.vector.reciprocal(rec8[:], den8[:])
                    nc.sync.dma_start(
                        rec_scr[pi].rearrange("two (a c) -> (two a) c", c=P),
                        rec8[:],
                    )
                    for par in (0, 1):
                        rcx = rc_pool.tile(
                            [1, QCH], F32, tag=f"rc{par}", name=f"rc{ft}{qh}{par}"
                        )
                        nc.sync.dma_start(rcx[:], rec_scr[pi, par : par + 1, :])
                        self.rc[par] = rcx

                def norm_item(par):
                    def run():
                        bcs = bcs_pool.tile(
                            [HD, QCH], F32, tag="bcs", name=f"bc{ft}{qh}{par}"
                        )
                        nc.gpsimd.partition_broadcast(bcs[:], self.rc[par][:])
                        with nc.allow_low_precision(reason="attn out cast"):
                            nc.vector.tensor_mul(
                                outP[ft][
                                    par * HD : (par + 1) * HD, q0 : q0 + QCH
                                ],
                                self.stg[par][:HD, :],
                                bcs[:],
                            )

                    return run

                return [recip_item, norm_item(0), norm_item(1)]

        wo_t = ys_pool = None

        def y_item(qt, wo_t, ys_pool):
            def run():
                for no in range(n_no):
                    yp = ps_mp.tile([P, QCH], F32, tag="mp", name=f"yp{qt}_{no}")
                    for ft in range(n_ft):
                        nc.tensor.matmul(
                            yp[:],
                            outP[ft][:, qt * P : (qt + 1) * P],
                            wo_t[ft][:, no * QCH : (no + 1) * QCH],
                            start=(ft == 0),
                            stop=(ft == n_ft - 1),
                        )
                    ys = ys_pool.tile([P, QCH], F32, tag="ys", name=f"ys{qt}_{no}")
                    nc.vector.tensor_copy(ys[:], yp[:])
                    nc.sync.dma_start(
                        y[qt * P : (qt + 1) * P, no * QCH : (no + 1) * QCH], ys[:]
                    )

            return run

        # ---------------- emission ----------------
        # upfront: chunk 0 loads + pair-0 projections, then start the (0,0)
        # attention pass incrementally as later chunks' keys become available
        qTp0, kTp0 = get_pair(0)
        att00 = MergedPass(0, 0)
        kts_per_ch = n_kt // n_ch
        # att00 only reads queries 0:512, so pair-0 q-projections for chunks
        # 1-3 defer out of the PE-saturated upfront into their (0,qh) passes
        q0_items = {}
        for ch in range(n_ch):
            upfront_loads(ch)
            # the chunk's k-projection must be EMITTED before the QKs that
            # read it — a read emitted first gets no dependency on the
            # later write and sees uninitialized SBUF
            if ch == 0:
                proj_item(0, ch, wq_t, qTp0, "q")()
            else:
                q0_items[ch] = proj_item(0, ch, wq_t, qTp0, "q")
            proj_item(0, ch, wk_t, kTp0, "k")()
            vi = [v_item(ch * n_sti + sti) for sti in range(n_sti)]
            att00.emit_kts(
                range(ch * kts_per_ch, (ch + 1) * kts_per_ch), sprinkle=vi, stride=1
            )
            for it in vi:
                it()
            if ch == 0:
                # later pairs' weights: deferred so chunk-0's x DMA + attention
                # start are not queued behind 3 MB of weight traffic
                for ft in range(1, n_ft):
                    load_w_pair(ft)

        # main loop, software-pipelined at pass boundaries: each pass emits
        # its first two k-tiles' QKs BEFORE the previous pass's finish, so the
        # PE fills the window where it used to sit behind PV(prev, kt15)
        # waiting on the last ACTIVATE, and the scalar engine never gaps
        rows_per_qh = n_st // n_qh
        prev = att00
        for ft in range(n_ft):
            last = ft == n_ft - 1
            if not last:
                proj_items = []
                for ch in range(n_ch):
                    qTn, kTn = get_pair(ft + 1)
                    proj_items.append(proj_item(ft + 1, ch, wq_t, qTn, "q"))
                    proj_items.append(proj_item(ft + 1, ch, wk_t, kTn, "k"))
            else:
                proj_items = []
                wo_pool = top.enter_context(tc.tile_pool(name="wo", bufs=1))
                ys_pool = top.enter_context(tc.tile_pool(name="ys", bufs=3))
                wo_t = [
                    wo_pool.tile([P, DO], BF16, tag=f"wo{f2}", name=f"wo{f2}")
                    for f2 in range(n_ft)
                ]
                for f2 in range(n_ft):
                    nc.gpsimd.dma_start(wo_t[f2][:], wout[f2 * P : (f2 + 1) * P, :])
            passes = [
                qh for qh in range(n_qh) if not (ft == 0 and qh == 0)
            ]
            for pi, qh in enumerate(passes):
                if ft == 0 and qh in q0_items:
                    # this pass's own q-projection must precede its QKs
                    q0_items.pop(qh)()
                a = MergedPass(ft, qh)
                a.emit_kts(range(0, 2))  # boundary-overlap group, no sprinkle
                prev.finish_stage1()
                work = prev.normalize_items()
                take = -(-len(proj_items) // (len(passes) - pi))
                work += proj_items[:take]
                proj_items = proj_items[take:]
                if last and qh > 0:
                    work += [
                        y_item(qt, wo_t, ys_pool)
                        for qt in range(
                            (qh - 1) * rows_per_qh, qh * rows_per_qh
                        )
                    ]
                stride = max(1, (n_kt - 2) // max(1, len(work))) if work else 1
                a.emit_kts(range(2, n_kt), sprinkle=work, stride=stride)
                for it in work:
                    it()
                prev = a
        # tail: finish the last pass with per-qt recip -> broadcast ->
        # multiply -> y chains so the final rows pipeline across engines
        prev.finish_stage1()
        for qi, qt in enumerate(
            range((n_qh - 1) * rows_per_qh, n_qh * rows_per_qh)
        ):
            qo = qi * P
            for par in (0, 1):
                rcx = rc_pool.tile(
                    [1, P], F32, tag=f"rt{par}", name=f"rt{qt}{par}"
                )
                nc.vector.reciprocal(
                    rcx[:], prev.stg[par][HD : HD + 1, qo : qo + P]
                )
                bcs = bcs_pool.tile(
                    [HD, P], F32, tag=f"bt{par}", name=f"bt{qt}{par}"
                )
                nc.gpsimd.partition_broadcast(bcs[:], rcx[:])
                with nc.allow_low_precision(reason="attn out cast"):
                    nc.vector.tensor_mul(
                        outP[n_ft - 1][
                            par * HD : (par + 1) * HD,
                            qt * P : (qt + 1) * P,
                        ],
                        prev.stg[par][:HD, qo : qo + P],
                        bcs[:],
                    )
            y_item(qt, wo_t, ys_pool)()

    nc.compile()
    nc._w_np_dtype = mybir.dt.np(BF16)
    nc._wo_np_dtype = mybir.dt.np(BF16)
    return nc


# problem sizes (hardcoded per contract)
B, S, D, H = 4, 2048, 1024, 16
DO = D
HN = H // 2  # heads per core
SCALE = (D // H) ** -0.5
N_CORES = 8

_NC_CACHE = None


def _get_nc():
    global _NC_CACHE
    if _NC_CACHE is None:
        _NC_CACHE = build_attention_v2(S, D, HN, DO, SCALE)
    return _NC_CACHE


def make_in_maps(x, w_qkv, w_out):
    """Shard full inputs into the 8 per-core input maps (weights cast to the
    dtype the compiled kernel's DRAM tensors expect)."""
    nc = _get_nc()
    wdt = getattr(nc, "_w_np_dtype", np.float32)
    wodt = getattr(nc, "_wo_np_dtype", np.float32)
    in_maps = []
    for c in range(N_CORES):
        b = c // 2
        cs = (c % 2) * HN * HD
        ce = cs + HN * HD
        in_maps.append(
            {
                "x": np.ascontiguousarray(x[b]),
                "wq": np.ascontiguousarray(w_qkv[:, cs:ce]).astype(wdt),
                "wk": np.ascontiguousarray(w_qkv[:, D + cs : D + ce]).astype(wdt),
                "wv": np.ascontiguousarray(w_qkv[:, 2 * D + cs : 2 * D + ce]).astype(
                    wdt
                ),
                "wout": np.ascontiguousarray(w_out[cs:ce, :]).astype(wodt),
            }
        )
    return in_maps


def combine_outputs(results, b_out):
    """Sum the two per-batch partials and add the bias."""
    y = np.empty((B, S, DO), dtype=np.float32)
    for b in range(B):
        y[b] = results[2 * b]["y"] + results[2 * b + 1]["y"] + b_out[None, :]
    return y


def kernel(x, w_qkv, w_out, b_out):
    x = np.asarray(x, dtype=np.float32)
    w_qkv = np.asarray(w_qkv, dtype=np.float32)
    w_out = np.asarray(w_out, dtype=np.float32)
    b_out = np.asarray(b_out, dtype=np.float32)
    nc = _get_nc()
    in_maps = make_in_maps(x, w_qkv, w_out)
    res = bass_utils.run_bass_kernel_spmd(nc, in_maps, core_ids=list(range(N_CORES)))
    return combine_outputs(res.results, b_out)
